# revision 1
# baseline (speedup 1.0000x reference)
"""MLA (multi-head latent attention) forward on 8 Trainium2 NeuronCores.

Sharding: tensor-parallel over heads (4 groups of 4 heads) x data-parallel
over batch (2), giving 8 cores. The latent kv_c / k_pe are computed on every
core (they are small); wq / wkv_b rows and wo columns are sharded by head.
Each core produces a partial [S, DIM] output (its heads' contribution through
wo); the host sums the 4 head-group partials per batch element.

Per-core dataflow is fully "transposed" (feature dims on SBUF partitions,
sequence on the free dim):
  qT = wq_g @ x^T            (nope rows, and rope rows split into lo/hi pairs)
  kv = x @ wkv_a^T           ([t, c] layout), RMS-normed, k_pe roped
  kv_cT, k_peT via PE transposes
  qprojT = wb_k^T-absorbed q (per head)
  scoresT[t, sq] accumulated per 128-t-tile; exp (no max subtraction --
    scores are O(1) here); causal masking by 0/1 multiply on diagonal tiles
  oT[c, sq] += kv_c[t-tile].T @ exp_scoresT ; l[sq] via ones-row matmul
  o_final = wb_v^T @ oT, normalized by broadcast(1/l) (PE K=1 matmul trick)
  out[s, :] = o_catT^T @ woT (partial over this core's heads)
"""

import numpy as np
import ml_dtypes

import concourse.bass as bass
import concourse.tile as tile
import concourse.mybir as mybir
from concourse import bass_utils

BF16 = mybir.dt.bfloat16
F32 = mybir.dt.float32
F32R = mybir.dt.float32r
AF = mybir.ActivationFunctionType
ALU = mybir.AluOpType
NPBF16 = ml_dtypes.bfloat16

B, S, DIM, H = 2, 2048, 2048, 16
NOPE, ROPE, VHD, KLR = 128, 64, 128, 512
QKHD = NOPE + ROPE
SCALE = QKHD ** -0.5
EPS = 1.1920929e-07
P = 128
_OUT_TILES = 16   # debug knob: how many output s-tiles to write
_QC_LIST = list(range(4))  # debug knob: which attention chunks to run
HG = 4            # heads per core
CH = 512          # sequence chunk (matmul free dim)
NCH = S // CH     # 4 chunks
NT = S // P       # 16 tiles of 128


def _emit(nc):
    dt = nc.dram_tensor
    xT = dt("xT", [P, NCH, NT, CH], BF16, kind="ExternalInput").ap()
    wqn = dt("wqn", [P, NT, 512], BF16, kind="ExternalInput").ap()
    wqlo = dt("wqlo", [P, NT, 128], BF16, kind="ExternalInput").ap()
    wqhi = dt("wqhi", [P, NT, 128], BF16, kind="ExternalInput").ap()
    wkva = dt("wkva", [P, NT, 576], BF16, kind="ExternalInput").ap()
    wbk = dt("wbk", [P, HG, KLR], BF16, kind="ExternalInput").ap()
    wbvT = dt("wbvT", [P, 4, HG, VHD], BF16, kind="ExternalInput").ap()
    woT = dt("woT", [P, 4, DIM], BF16, kind="ExternalInput").ap()
    cosS = dt("cosS", [P, S], BF16, kind="ExternalInput").ap()
    sinS = dt("sinS", [P, S], BF16, kind="ExternalInput").ap()
    xkv = dt("xkv", [P, NT, CH], BF16, kind="ExternalInput").ap()
    cosTk = dt("cosTk", [P, 4, 32], BF16, kind="ExternalInput").ap()
    sinTk = dt("sinTk", [P, 4, 32], BF16, kind="ExternalInput").ap()
    mmst = dt("mmst", [P, 896], BF16, kind="ExternalInput").ap()
    onec = dt("onec", [P, 1], BF16, kind="ExternalInput").ap()
    oner = dt("oner", [1, P], F32R, kind="ExternalInput").ap()
    ident = dt("ident", [P, P], BF16, kind="ExternalInput").ap()
    epsb = dt("epsb", [P, 1], F32, kind="ExternalInput").ap()
    outp = dt("outp", [P, NT, DIM], F32, kind="ExternalOutput").ap()

    with tile.TileContext(nc) as tc:
        from contextlib import ExitStack

        with ExitStack() as ctx:
            ec = ctx.enter_context
            const = ec(tc.tile_pool(name="const", bufs=1))
            xpool = ec(tc.tile_pool(name="xpool", bufs=2))
            qpp = ec(tc.tile_pool(name="qpp", bufs=5))
            ocp = ec(tc.tile_pool(name="ocp", bufs=2))
            expp = ec(tc.tile_pool(name="expp", bufs=3))
            ovp = ec(tc.tile_pool(name="ovp", bufs=6))
            f32p = ec(tc.tile_pool(name="f32p", bufs=3))
            smallp = ec(tc.tile_pool(name="smallp", bufs=7))
            statp = ec(tc.tile_pool(name="statp", bufs=4))

            psA = ec(tc.tile_pool(name="psA", bufs=4, space="PSUM"))
            psB = ec(tc.tile_pool(name="psB", bufs=2, space="PSUM"))
            psL = ec(tc.tile_pool(name="psL", bufs=1, space="PSUM"))
            psC = ec(tc.tile_pool(name="psC", bufs=1, space="PSUM"))

            # ---- resident weights/tables ----
            # First-needed data first: x chunk 0 + wqn, interleaved per k-slice
            # so the k=0 projection group unblocks after ~256KB of DMA.
            wqn_sb = const.tile([P, NT, 512], BF16, tag="wqn")
            x0_sb = xpool.tile([P, NT, CH], BF16, tag="x")
            for k in range(NT):
                nc.sync.dma_start(x0_sb[:, k, :], xT[:, 0, k, :])
                nc.sync.dma_start(wqn_sb[:, k, :], wqn[:, k, :])
            # rope-row weights go via the qp pool (slots free up for qproj later)
            wqlo_sb = qpp.tile([P, NT, 128], BF16, tag="qp")
            nc.sync.dma_start(wqlo_sb[:], wqlo)
            wqhi_sb = qpp.tile([P, NT, 128], BF16, tag="qp")
            nc.sync.dma_start(wqhi_sb[:], wqhi)
            cosS_sb = qpp.tile([P, S], BF16, tag="qp")
            nc.sync.dma_start(cosS_sb[:], cosS)
            sinS_sb = qpp.tile([P, S], BF16, tag="qp")
            nc.sync.dma_start(sinS_sb[:], sinS)

            xkv_sb = const.tile([P, NT, CH], BF16, tag="xkv")
            for k in range(NT):
                nc.sync.dma_start(xkv_sb[:, k, :], xkv[:, k, :])
            wkva_sb = const.tile([P, NT, 576], BF16, tag="wkva")
            for k in range(NT):
                nc.sync.dma_start(wkva_sb[:, k, :], wkva[:, k, :])
            wbk_sb = const.tile([P, HG, KLR], BF16, tag="wbk")
            nc.sync.dma_start(wbk_sb[:], wbk)
            wbvT_sb = const.tile([P, 4, HG, VHD], BF16, tag="wbvT")
            nc.sync.dma_start(wbvT_sb[:], wbvT)
            cosTk_sb = const.tile([P, 4, 32], BF16, tag="cosTk")
            nc.sync.dma_start(cosTk_sb[:], cosTk)
            sinTk_sb = const.tile([P, 4, 32], BF16, tag="sinTk")
            nc.sync.dma_start(sinTk_sb[:], sinTk)
            mmst_sb = const.tile([P, 896], BF16, tag="mmst")
            nc.sync.dma_start(mmst_sb[:], mmst)
            onec_sb = const.tile([P, 1], BF16, tag="onec")
            nc.sync.dma_start(onec_sb[:], onec)
            oner_sb = const.tile([1, P], F32R, tag="oner")
            nc.sync.dma_start(oner_sb[:], oner)
            ident_sb = const.tile([P, P], BF16, tag="ident")
            nc.sync.dma_start(ident_sb[:], ident)
            eps_sb = const.tile([P, 1], F32, tag="epsb")
            nc.sync.dma_start(eps_sb[:], epsb)

            # ---- persistent activations ----
            qnope_sb = const.tile([P, HG, S], BF16, tag="qnope")
            # head h rope operand: partitions (h%2)*64 + [0,64), index h//2
            qpe_sb = const.tile([P, 2, S], BF16, tag="qpe")
            kvc_sb = const.tile([P, NT, KLR], BF16, tag="kvc")
            kvcT_sb = const.tile([P, 4, S], BF16, tag="kvcT")
            kpeT_sb = const.tile([P, S], BF16, tag="kpeT")  # dup rows 64:128
            krop_sb = const.tile([P, NT, 64], BF16, tag="krop")

            # ---- local kv projection: this core's chunk only (4 s-tiles) ----
            # results are exchanged via AllGather across the 4 head-group
            # cores of the batch (the "MLA advantage": latent kv is tiny)
            dramp = ec(tc.tile_pool(name="dramp", bufs=1, space="DRAM"))
            cc_in = dramp.tile([P, 4, 576], BF16, tag="ccin")
            cc_out = dramp.tile([4, P, 4, 576], BF16, tag="ccout")
            kvcl = ocp.tile([P, 4, KLR], BF16, tag="oc")
            kper = smallp.tile([P, 4, 64], BF16, tag="sm")
            for st2 in range(4):
                ps_c = psB.tile([P, KLR], F32, tag="mm")
                ps_r = psL.tile([P, 64], F32, tag="lacc")
                for k in range(NT):
                    nc.tensor.matmul(
                        ps_c,
                        lhsT=xkv_sb[:, k, st2 * 128 : (st2 + 1) * 128],
                        rhs=wkva_sb[:, k, 0:512],
                        start=(k == 0),
                        stop=(k == NT - 1),
                    )
                for k in range(NT):
                    nc.tensor.matmul(
                        ps_r,
                        lhsT=xkv_sb[:, k, st2 * 128 : (st2 + 1) * 128],
                        rhs=wkva_sb[:, k, 512:576],
                        start=(k == 0),
                        stop=(k == NT - 1),
                    )
                # RMS norm over c
                scr = f32p.tile([P, KLR], F32, tag="f32")
                ssq = statp.tile([P, 1], F32, tag="st")
                nc.scalar.activation(scr[:], ps_c, AF.Square, accum_out=ssq[:])
                rms = statp.tile([P, 1], F32, tag="st")
                nc.scalar.activation(
                    rms[:], ssq[:], AF.Sqrt, bias=eps_sb[:], scale=1.0 / KLR
                )
                rin = statp.tile([P, 1], F32, tag="st")
                nc.vector.reciprocal(rin[:], rms[:])
                nc.vector.tensor_scalar_mul(kvcl[:, st2, :], ps_c, rin[:])
                nc.vector.tensor_copy(kper[:, st2, :], ps_r)
            # k_pe rope (local chunk, [t, r] layout)
            u1 = smallp.tile([P, 4, 32], BF16, tag="sm4")
            u2 = smallp.tile([P, 4, 32], BF16, tag="sm4")
            nc.vector.tensor_tensor(u1[:], kper[:, :, 0:32], cosTk_sb[:], ALU.mult)
            nc.vector.tensor_tensor(u2[:], kper[:, :, 32:64], sinTk_sb[:], ALU.mult)
            krl = smallp.tile([P, 4, 64], BF16, tag="sm")
            nc.vector.tensor_tensor(krl[:, :, 0:32], u1[:], u2[:], ALU.subtract)
            u3 = smallp.tile([P, 4, 32], BF16, tag="sm4")
            u4 = smallp.tile([P, 4, 32], BF16, tag="sm4")
            nc.vector.tensor_tensor(u3[:], kper[:, :, 0:32], sinTk_sb[:], ALU.mult)
            nc.vector.tensor_tensor(u4[:], kper[:, :, 32:64], cosTk_sb[:], ALU.mult)
            nc.vector.tensor_tensor(krl[:, :, 32:64], u3[:], u4[:], ALU.add)
            # ship local kv to peers
            nc.gpsimd.dma_start(cc_in[:, :, 0:512], kvcl[:])
            nc.gpsimd.dma_start(cc_in[:, :, 512:576], krl[:])
            nc.gpsimd.collective_compute(
                "AllGather",
                ALU.bypass,
                replica_groups=[[0, 1, 2, 3], [4, 5, 6, 7]],
                ins=[cc_in.opt()],
                outs=[cc_out.opt()],
            )

            # gather-back: pull peers' kv chunks into the full kv tensors
            for gg in range(4):
                sl = slice(gg * 4, (gg + 1) * 4)
                nc.sync.dma_start(kvc_sb[:, sl, :], cc_out[gg, :, :, 0:512])
                nc.sync.dma_start(krop_sb[:, sl, :], cc_out[gg, :, :, 512:576])
            for st in range(NT):
                tsl = slice(st * 128, (st + 1) * 128)
                for cs in range(4):
                    pool_t = psC if cs % 2 == 0 else psL
                    tp = pool_t.tile([P, P], BF16, tag="aux" if cs % 2 == 0 else "lacc")
                    nc.tensor.transpose(
                        tp, kvc_sb[:, st, cs * 128 : (cs + 1) * 128], ident_sb[:]
                    )
                    nc.vector.tensor_copy(kvcT_sb[:, cs, tsl], tp)
                kp = psC.tile([64, P], BF16, tag="aux")
                nc.tensor.transpose(kp, krop_sb[:, st, :], ident_sb[:])
                nc.vector.tensor_copy(kpeT_sb[0:64, tsl], kp)
            # duplicate k_peT into partitions 64:128 (for heads at base 64)
            nc.sync.dma_start(kpeT_sb[64:128, :], kpeT_sb[0:64, :])

            # ================= phase 1: projections =================
            for q in range(NCH):
                qs = slice(q * CH, (q + 1) * CH)
                if q == 0:
                    x_sb = x0_sb
                else:
                    x_sb = xpool.tile([P, NT, CH], BF16, tag="x")
                    nc.sync.dma_start(x_sb[:], xT[:, q])

                # q projections, k-outer: 6 m-tiles accumulate in parallel
                # (4 nope head tiles + rope lo + rope hi)
                lo_t = smallp.tile([P, CH], BF16, tag="sm")
                hi_t = smallp.tile([P, CH], BF16, tag="sm")
                qps = [
                    psB.tile([P, CH], F32, tag="mm", name=f"qt{m}") for m in range(2)
                ] + [
                    psA.tile([P, CH], F32, tag="oacc", name=f"qt{m + 2}")
                    for m in range(4)
                ]
                for k in range(NT):
                    for m in range(HG):
                        nc.tensor.matmul(
                            qps[m],
                            lhsT=wqn_sb[:, k, m * 128 : (m + 1) * 128],
                            rhs=x_sb[:, k, :],
                            start=(k == 0),
                            stop=(k == NT - 1),
                        )
                    nc.tensor.matmul(
                        qps[4],
                        lhsT=wqlo_sb[:, k, :],
                        rhs=x_sb[:, k, :],
                        start=(k == 0),
                        stop=(k == NT - 1),
                    )
                    nc.tensor.matmul(
                        qps[5],
                        lhsT=wqhi_sb[:, k, :],
                        rhs=x_sb[:, k, :],
                        start=(k == 0),
                        stop=(k == NT - 1),
                    )
                for m in range(HG):
                    nc.vector.tensor_copy(qnope_sb[:, m, qs], qps[m])
                nc.vector.tensor_copy(lo_t[:], qps[4])
                nc.vector.tensor_copy(hi_t[:], qps[5])
                # rope on full-width tiles
                t1 = smallp.tile([P, CH], BF16, tag="sm")
                t2 = smallp.tile([P, CH], BF16, tag="sm")
                nc.vector.tensor_tensor(t1[:], lo_t[:], cosS_sb[:, qs], ALU.mult)
                nc.vector.tensor_tensor(t2[:], hi_t[:], sinS_sb[:, qs], ALU.mult)
                nc.vector.tensor_tensor(t1[:], t1[:], t2[:], ALU.subtract)
                t3 = smallp.tile([P, CH], BF16, tag="sm")
                t4 = smallp.tile([P, CH], BF16, tag="sm")
                nc.vector.tensor_tensor(t3[:], lo_t[:], sinS_sb[:, qs], ALU.mult)
                nc.vector.tensor_tensor(t4[:], hi_t[:], cosS_sb[:, qs], ALU.mult)
                nc.vector.tensor_tensor(t3[:], t3[:], t4[:], ALU.add)
                # repack to per-head contiguous [lo;hi] via SBUF->SBUF DMA
                for hh in range(HG):
                    base = (hh % 2) * 64
                    j = hh // 2
                    nc.sync.dma_start(
                        qpe_sb[base : base + 32, j, qs], t1[hh * 32 : (hh + 1) * 32, :]
                    )
                    nc.sync.dma_start(
                        qpe_sb[base + 32 : base + 64, j, qs],
                        t3[hh * 32 : (hh + 1) * 32, :],
                    )

            # wo loads late, reusing the x-chunk slots
            woT_sb = xpool.tile([P, 4, DIM], BF16, tag="x")
            nc.sync.dma_start(woT_sb[:], woT)

            # ================= phase 2: attention =================
            for qc in _QC_LIST:
                qs = slice(qc * CH, (qc + 1) * CH)
                ocat = ocp.tile([P, HG, CH], BF16, tag="oc")
                for hh in range(HG):
                    # absorbed query projection for (head, chunk)
                    qp = qpp.tile([P, 4, CH], BF16, tag="qp")
                    for cs in range(4):
                        ps = psB.tile([P, CH], F32, tag="mm")
                        nc.tensor.matmul(
                            ps,
                            lhsT=wbk_sb[:, hh, cs * 128 : (cs + 1) * 128],
                            rhs=qnope_sb[:, hh, qs],
                            start=True,
                            stop=True,
                        )
                        nc.vector.tensor_copy(qp[:, cs, :], ps)
                    base = (hh % 2) * 64
                    jj = hh // 2
                    q_pe = qpe_sb[base : base + 64, jj, qs]
                    k_pe = kpeT_sb[base : base + 64, :]

                    oacc = [
                        psA.tile([P, CH], F32, tag="oacc", name=f"oacc{i}")
                        for i in range(4)
                    ]
                    l_ps = psL.tile([1, CH], F32, tag="lacc")
                    nti = 4 * qc + 4
                    for ti in range(nti):
                        tsl = slice(ti * 128, (ti + 1) * 128)
                        off = max(0, ti * 128 - qc * CH)
                        nw = CH - off  # live sq columns (diag tiles shrink)
                        sc = psB.tile([P, CH], F32, tag="mm")
                        for cs in range(4):
                            nc.tensor.matmul(
                                sc[:, :nw],
                                lhsT=kvcT_sb[:, cs, tsl],
                                rhs=qp[:, cs, off:],
                                start=(cs == 0),
                                stop=False,
                            )
                        nc.tensor.matmul(
                            sc[:, :nw],
                            lhsT=k_pe[:, tsl],
                            rhs=q_pe[:, off:],
                            start=False,
                            stop=True,
                        )
                        ex = expp.tile([P, CH], BF16, tag="exp")
                        nc.scalar.activation(ex[:, :nw], sc[:, :nw], AF.Exp)
                        if ti * 128 - qc * CH >= 0:  # diagonal: causal 0/1 mask
                            nc.vector.tensor_tensor(
                                ex[:, :nw],
                                ex[:, :nw],
                                mmst_sb[:, 384 : 384 + nw],
                                ALU.mult,
                            )
                        first, last = (ti == 0), (ti == nti - 1)
                        for cs in range(4):
                            nc.tensor.matmul(
                                oacc[cs][:, off:],
                                lhsT=kvc_sb[:, ti, cs * 128 : (cs + 1) * 128],
                                rhs=ex[:, :nw],
                                start=first,
                                stop=last,
                            )
                        nc.tensor.matmul(
                            l_ps[:, off:],
                            lhsT=onec_sb[:],
                            rhs=ex[:, :nw],
                            start=first,
                            stop=last,
                        )
                    # 1/l broadcast across partitions via K=1 matmul
                    rl = f32p.tile([1, CH], F32R, tag="f32")
                    with nc.allow_low_precision(reason="1/l bcast via f32r matmul"):
                        nc.vector.reciprocal(rl[:], l_ps)
                    bc_ps = psC.tile([P, CH], F32, tag="aux")
                    nc.tensor.matmul(
                        bc_ps, lhsT=oner_sb[:], rhs=rl[:], start=True, stop=True
                    )
                    bc = f32p.tile([P, CH], F32, tag="f32")
                    nc.scalar.copy(bc[:], bc_ps)
                    # value up-projection
                    ov = [
                        ovp.tile([P, CH], BF16, tag="ov", name=f"ov{i}")
                        for i in range(4)
                    ]
                    for cs in range(4):
                        if cs % 2 == 0:
                            nc.vector.tensor_copy(ov[cs][:], oacc[cs])
                        else:
                            nc.scalar.copy(ov[cs][:], oacc[cs])
                    of_ps = psC.tile([P, CH], F32, tag="aux")
                    for cs in range(4):
                        nc.tensor.matmul(
                            of_ps,
                            lhsT=wbvT_sb[:, cs, hh, :],
                            rhs=ov[cs][:],
                            start=(cs == 0),
                            stop=(cs == 3),
                        )
                    nc.vector.tensor_tensor(ocat[:, hh, :], of_ps, bc[:], ALU.mult)
                # ---- output projection for this chunk ----
                for st2 in range(4):
                    st = qc * 4 + st2
                    if st >= _OUT_TILES:
                        continue
                    for dc in range(4):
                        op = psB.tile([P, CH], F32, tag="mm")
                        for es in range(4):
                            nc.tensor.matmul(
                                op,
                                lhsT=ocat[:, es, st2 * 128 : (st2 + 1) * 128],
                                rhs=woT_sb[:, es, dc * CH : (dc + 1) * CH],
                                start=(es == 0),
                                stop=(es == 3),
                            )
                        ot = f32p.tile([P, CH], F32, tag="f32")
                        nc.vector.tensor_copy(ot[:], op)
                        nc.sync.dma_start(outp[:, st, dc * CH : (dc + 1) * CH], ot[:])
    return nc


# --- walrus in this container rejects >1 sem-wait per instruction; split ---
def _split_excess_waits(nc, max_waits=1):
    for f in nc.m.functions:
        for bb in f.blocks:
            if not any(
                i.sync_info is not None and len(i.sync_info.on_wait) > max_waits
                for i in bb.instructions
            ):
                continue
            new_insts = []
            for inst in bb.instructions:
                si = inst.sync_info
                if si is not None and len(si.on_wait) > max_waits:
                    waits = list(si.on_wait)
                    extra, keep = waits[:-max_waits], waits[-max_waits:]
                    for j in range(0, len(extra), max_waits):
                        nop = mybir.InstNoOp(
                            name=f"{inst.name}-wsplit-{j}", ins=[], outs=[]
                        )
                        nop.engine = inst.engine
                        nop.sync_info = mybir.SyncInfo(
                            on_wait=extra[j : j + max_waits], on_update=[]
                        )
                        new_insts.append(nop)
                    inst.sync_info = mybir.SyncInfo(
                        on_wait=keep, on_update=list(si.on_update)
                    )
                new_insts.append(inst)
            bb.instructions = new_insts


_NC = None


def _module():
    global _NC
    if _NC is None:
        nc = bass.Bass(
            "TRN2", target_bir_lowering=False, debug=False, num_devices=8
        )
        _emit(nc)
        _split_excess_waits(nc)
        _NC = nc
    return _NC


def _prep_core(core, x, wq, wkv_a, kv_norm_w, wkv_b, wo, fc, fs):
    """Build the per-core input map (numpy, host-side sharding + layouts)."""
    b, g = core // 4, core % 4
    heads = [4 * g + i for i in range(HG)]

    def bf(a):
        return np.ascontiguousarray(a.astype(NPBF16))

    m = {}
    xx = x[b]  # [S, DIM]
    m["xT"] = bf(xx.reshape(NCH, CH, NT, P).transpose(3, 0, 2, 1))

    rows_n = np.concatenate([h * QKHD + np.arange(NOPE) for h in heads])
    rows_lo = np.concatenate([h * QKHD + NOPE + 2 * np.arange(32) for h in heads])
    rows_hi = np.concatenate([h * QKHD + NOPE + 2 * np.arange(32) + 1 for h in heads])
    wqs = (wq * SCALE).astype(np.float32)
    for nm, rows in (("wqn", rows_n), ("wqlo", rows_lo), ("wqhi", rows_hi)):
        sel = wqs[rows]  # [M, DIM]
        m[nm] = bf(sel.T.reshape(NT, P, len(rows)).transpose(1, 0, 2))

    krows = np.concatenate(
        [np.arange(KLR), KLR + 2 * np.arange(32), KLR + 2 * np.arange(32) + 1]
    )
    m["wkva"] = bf(wkv_a[krows].T.reshape(NT, P, 576).transpose(1, 0, 2))

    wb = wkv_b.reshape(H, NOPE + VHD, KLR)
    wk = wb[heads, :NOPE, :] * kv_norm_w[None, None, :]  # [HG, d, c]
    m["wbk"] = bf(wk.transpose(1, 0, 2))  # [p=d, hh, c]
    wv = wb[heads, NOPE:, :] * kv_norm_w[None, None, :]  # [HG, d, c]
    m["wbvT"] = bf(wv.transpose(2, 0, 1).reshape(4, P, HG, VHD).transpose(1, 0, 2, 3))

    wo_s = wo[:, 4 * g * VHD : 4 * (g + 1) * VHD]  # [DIM, 512]
    m["woT"] = bf(wo_s.T.reshape(4, P, DIM).transpose(1, 0, 2))

    m["cosS"] = bf(np.tile(fc.T, (4, 1)))
    m["sinS"] = bf(np.tile(fs.T, (4, 1)))
    ct = fc.reshape(NT, P, 32).transpose(1, 0, 2)
    st_ = fs.reshape(NT, P, 32).transpose(1, 0, 2)
    m["cosTk"] = bf(ct[:, 4 * g : 4 * (g + 1), :])
    m["sinTk"] = bf(st_[:, 4 * g : 4 * (g + 1), :])
    m["xkv"] = np.ascontiguousarray(m["xT"][:, g])

    pp = np.arange(P)[:, None]
    uu = np.arange(896)[None, :]
    m["mmst"] = bf((pp <= uu - 384).astype(np.float32))
    m["onec"] = bf(np.ones((P, 1), np.float32))
    m["oner"] = np.ones((1, P), np.float32)
    m["ident"] = bf(np.eye(P, dtype=np.float32))
    m["epsb"] = np.full((P, 1), EPS, np.float32)
    return m


def _make_in_maps(inputs):
    x = np.asarray(inputs["x"], np.float32)
    wq = np.asarray(inputs["wq"], np.float32)
    wkv_a = np.asarray(inputs["wkv_a"], np.float32)
    kv_norm_w = np.asarray(inputs["kv_norm_w"], np.float32)
    wkv_b = np.asarray(inputs["wkv_b"], np.float32)
    wo = np.asarray(inputs["wo"], np.float32)
    fc = np.asarray(inputs["freqs_cos"], np.float32)
    fs = np.asarray(inputs["freqs_sin"], np.float32)
    return [
        _prep_core(c, x, wq, wkv_a, kv_norm_w, wkv_b, wo, fc, fs) for c in range(8)
    ]


def _assemble(results):
    out = np.zeros((B, S, DIM), np.float32)
    for c in range(8):
        b = c // 4
        part = results[c]["outp"]  # [P, NT, DIM]
        out[b] += part.transpose(1, 0, 2).reshape(S, DIM)
    return out


def kernel(**inputs):
    nc = _module()
    in_maps = _make_in_maps(inputs)
    res = bass_utils.run_bass_kernel_spmd(nc, in_maps, core_ids=list(range(8)))
    return _assemble(res.results)



# revision 8
# speedup vs baseline: 1.1058x; 1.1058x over previous
"""MLA (multi-head latent attention) forward on 8 Trainium2 NeuronCores.

Sharding: tensor-parallel over heads (4 groups of 4 heads) x data-parallel
over batch (2), giving 8 cores. Every core computes the (small) latent
kv_c / k_pe projection over the full sequence itself — no collectives, no
cross-core coupling. wq / wkv_b rows and wo columns are sharded by head.
Each core produces a partial [S, DIM] output (its heads' contribution
through wo); the host sums the 4 head-group partials per batch element.

Key restructure vs a direct port of the reference: absorption is done on
the K/V side. Per head we precompute
    k_abs  = wbk_h @ kv_c^T        [d=128, S]
    v_absT = kv_c @ wbv_h^T        [S, d=128]
so the score matmul contracts over d=128 (1 matmul / 128-key-tile) and the
o accumulation contracts over the key tile (1 matmul), instead of
contracting over the 512-wide latent each time.

Per-core dataflow is "transposed" (feature dims on SBUF partitions,
sequence on the free dim):
  phase A (per 512-seq chunk): qT = wq_g @ x^T; kv = x @ wkv_a^T with
    RMS sum-of-squares accumulated (ACT does only Square); k_peT computed
    directly transposed and roped in [r, t] layout.
  phase B: rsqrt of mean-square (one ACT Sqrt + one DVE reciprocal for all
    tiles), normalize kv_c, PE-transpose to kv_cT, build k_abs / v_absT.
  phase C (per chunk, per head): scoresT[t, sq] = k_abs^T q_nope +
    k_peT^T q_pe; exp (no max subtraction — scores are O(1)); causal 0/1
    mask on diagonal tiles; oacc[d, sq] += v_absT^T exp; l via ones-row
    matmul; normalize by broadcast(1/l); out projection through woT.
"""

import numpy as np
import ml_dtypes

import concourse.bass as bass
import concourse.tile as tile
import concourse.mybir as mybir
from concourse import bass_utils

BF16 = mybir.dt.bfloat16
F32 = mybir.dt.float32
F32R = mybir.dt.float32r
AF = mybir.ActivationFunctionType
ALU = mybir.AluOpType
NPBF16 = ml_dtypes.bfloat16

B, S, DIM, H = 2, 2048, 2048, 16
NOPE, ROPE, VHD, KLR = 128, 64, 128, 512
QKHD = NOPE + ROPE
SCALE = QKHD ** -0.5
EPS = 1.1920929e-07
P = 128
HG = 4            # heads per core
CH = 512          # sequence chunk (matmul free dim)
NCH = S // CH     # 4 chunks
NT = S // P       # 16 tiles of 128


def _emit(nc):
    dt = nc.dram_tensor
    xT = dt("xT", [P, NCH, NT, CH], BF16, kind="ExternalInput").ap()
    wqn = dt("wqn", [P, NT, 512], BF16, kind="ExternalInput").ap()
    wqlo = dt("wqlo", [P, NT, 128], BF16, kind="ExternalInput").ap()
    wqhi = dt("wqhi", [P, NT, 128], BF16, kind="ExternalInput").ap()
    wkva = dt("wkva", [P, NT, 576], BF16, kind="ExternalInput").ap()
    wbkT = dt("wbkT", [P, 4, HG, NOPE], BF16, kind="ExternalInput").ap()
    wbvT = dt("wbvT", [P, 4, HG * VHD], BF16, kind="ExternalInput").ap()
    woT = dt("woT", [P, 4, DIM], BF16, kind="ExternalInput").ap()
    cosS = dt("cosS", [P, S], BF16, kind="ExternalInput").ap()
    sinS = dt("sinS", [P, S], BF16, kind="ExternalInput").ap()
    mmst = dt("mmst", [P, 896], BF16, kind="ExternalInput").ap()
    onec = dt("onec", [P, 1], BF16, kind="ExternalInput").ap()
    oner = dt("oner", [1, P], F32R, kind="ExternalInput").ap()
    ident = dt("ident", [P, P], BF16, kind="ExternalInput").ap()
    epsb = dt("epsb", [P, 1], F32, kind="ExternalInput").ap()
    outp = dt("outp", [P, NT, DIM], BF16, kind="ExternalOutput").ap()

    with tile.TileContext(nc) as tc:
        from contextlib import ExitStack

        with ExitStack() as ctx:
            ec = ctx.enter_context
            const = ec(tc.tile_pool(name="const", bufs=1))
            # 1-buf pools whose space is reused by a phase-B tensor after the
            # phase-A weight's last read
            wqpool = ec(tc.tile_pool(name="wqpool", bufs=1))
            wkpool = ec(tc.tile_pool(name="wkpool", bufs=1))
            xpool = ec(tc.tile_pool(name="xpool", bufs=2))
            ocp = ec(tc.tile_pool(name="ocp", bufs=2))
            expp = ec(tc.tile_pool(name="expp", bufs=3))
            f32p = ec(tc.tile_pool(name="f32p", bufs=3))
            smallp = ec(tc.tile_pool(name="smallp", bufs=7))

            psA = ec(tc.tile_pool(name="psA", bufs=4, space="PSUM"))
            psB = ec(tc.tile_pool(name="psB", bufs=2, space="PSUM"))
            psL = ec(tc.tile_pool(name="psL", bufs=1, space="PSUM"))
            psC = ec(tc.tile_pool(name="psC", bufs=1, space="PSUM"))

            # ---- resident weights/tables ----
            # First-needed data first: x chunk 0 + wqn, interleaved per
            # k-slice so the k=0 projection group unblocks early.
            wqn_sb = wqpool.tile([P, NT, 512], BF16, tag="wq")
            x0_sb = xpool.tile([P, NT, CH], BF16, tag="x")
            for k in range(NT):
                nc.sync.dma_start(x0_sb[:, k, :], xT[:, 0, k, :])
                nc.sync.dma_start(wqn_sb[:, k, :], wqn[:, k, :])
            wqlo_sb = const.tile([P, NT, 128], BF16, tag="wqlo")
            nc.sync.dma_start(wqlo_sb[:], wqlo)
            wqhi_sb = const.tile([P, NT, 128], BF16, tag="wqhi")
            nc.sync.dma_start(wqhi_sb[:], wqhi)
            cosS_sb = const.tile([P, S], BF16, tag="cosS")
            nc.sync.dma_start(cosS_sb[:], cosS)
            sinS_sb = const.tile([P, S], BF16, tag="sinS")
            nc.sync.dma_start(sinS_sb[:], sinS)
            wkva_sb = wkpool.tile([P, NT, 576], BF16, tag="wk")
            for k in range(NT):
                nc.sync.dma_start(wkva_sb[:, k, :], wkva[:, k, :])
            wbkT_sb = const.tile([P, 4, HG, NOPE], BF16, tag="wbkT")
            nc.sync.dma_start(wbkT_sb[:], wbkT)
            wbvT_sb = const.tile([P, 4, HG * VHD], BF16, tag="wbvT")
            nc.sync.dma_start(wbvT_sb[:], wbvT)
            mmst_sb = const.tile([P, 896], BF16, tag="mmst")
            nc.sync.dma_start(mmst_sb[:], mmst)
            onec_sb = const.tile([P, 1], BF16, tag="onec")
            nc.sync.dma_start(onec_sb[:], onec)
            oner_sb = const.tile([1, P], F32R, tag="oner")
            nc.sync.dma_start(oner_sb[:], oner)
            ident_sb = const.tile([P, P], BF16, tag="ident")
            nc.sync.dma_start(ident_sb[:], ident)
            eps_sb = const.tile([P, 1], F32, tag="epsb")
            nc.sync.dma_start(eps_sb[:], epsb)

            # ---- persistent activations ----
            qnope_sb = const.tile([P, HG, S], BF16, tag="qnope")
            # head h rope operand: partitions (h%2)*64 + [0,64), index h//2
            qpe_sb = const.tile([P, 2, S], BF16, tag="qpe")
            kvcu_sb = const.tile([P, NT, KLR], BF16, tag="kvcu")  # un-normed
            ssqs_sb = const.tile([P, NT], F32, tag="ssqs")
            kvcT_sb = const.tile([P, 4, S], BF16, tag="kvcT")
            kpeT_sb = const.tile([P, S], BF16, tag="kpeT")  # dup rows 64:128
            # kabs / vabsT are allocated in phase B from the wq / wkva pools
            # (same space, reused after the projection weights' last read)

            # ================= phase A: projections =================
            for q in range(NCH):
                qs = slice(q * CH, (q + 1) * CH)
                if q == 0:
                    x_sb = x0_sb
                else:
                    x_sb = xpool.tile([P, NT, CH], BF16, tag="x")
                    nc.sync.dma_start(x_sb[:], xT[:, q])

                # q projections, k-outer: 6 m-tiles accumulate in parallel
                # (4 nope head tiles + rope lo + rope hi)
                lo_t = smallp.tile([P, CH], BF16, tag="sm")
                hi_t = smallp.tile([P, CH], BF16, tag="sm")
                qps = [
                    psA.tile([P, CH], F32, tag="oacc", name=f"qt{m}")
                    for m in range(HG)
                ] + [
                    psB.tile([P, CH], F32, tag="mm", name=f"qr{m}")
                    for m in range(2)
                ]
                for k in range(NT):
                    for m in range(HG):
                        nc.tensor.matmul(
                            qps[m],
                            lhsT=wqn_sb[:, k, m * 128 : (m + 1) * 128],
                            rhs=x_sb[:, k, :],
                            start=(k == 0),
                            stop=(k == NT - 1),
                        )
                    nc.tensor.matmul(
                        qps[4],
                        lhsT=wqlo_sb[:, k, :],
                        rhs=x_sb[:, k, :],
                        start=(k == 0),
                        stop=(k == NT - 1),
                    )
                    nc.tensor.matmul(
                        qps[5],
                        lhsT=wqhi_sb[:, k, :],
                        rhs=x_sb[:, k, :],
                        start=(k == 0),
                        stop=(k == NT - 1),
                    )
                for m in range(HG):
                    nc.vector.tensor_copy(qnope_sb[:, m, qs], qps[m])
                nc.vector.tensor_copy(lo_t[:], qps[4])
                nc.vector.tensor_copy(hi_t[:], qps[5])
                # q rope on full-width tiles
                t1 = smallp.tile([P, CH], BF16, tag="sm")
                t2 = smallp.tile([P, CH], BF16, tag="sm")
                nc.vector.tensor_tensor(t1[:], lo_t[:], cosS_sb[:, qs], ALU.mult)
                nc.vector.tensor_tensor(t2[:], hi_t[:], sinS_sb[:, qs], ALU.mult)
                nc.vector.tensor_tensor(t1[:], t1[:], t2[:], ALU.subtract)
                t3 = smallp.tile([P, CH], BF16, tag="sm")
                t4 = smallp.tile([P, CH], BF16, tag="sm")
                nc.vector.tensor_tensor(t3[:], lo_t[:], sinS_sb[:, qs], ALU.mult)
                nc.vector.tensor_tensor(t4[:], hi_t[:], cosS_sb[:, qs], ALU.mult)
                nc.vector.tensor_tensor(t3[:], t3[:], t4[:], ALU.add)
                # repack to per-head contiguous [re;im] via SBUF->SBUF DMA
                for hh in range(HG):
                    base = (hh % 2) * 64
                    j = hh // 2
                    nc.sync.dma_start(
                        qpe_sb[base : base + 32, j, qs], t1[hh * 32 : (hh + 1) * 32, :]
                    )
                    nc.sync.dma_start(
                        qpe_sb[base + 32 : base + 64, j, qs],
                        t3[hh * 32 : (hh + 1) * 32, :],
                    )

                # kv_c projection for this chunk's 4 t-tiles ([t, c] layout)
                for st2 in range(4):
                    ti = q * 4 + st2
                    ps_c = psA.tile([P, KLR], F32, tag="oacc")
                    for k in range(NT):
                        nc.tensor.matmul(
                            ps_c,
                            lhsT=x_sb[:, k, st2 * 128 : (st2 + 1) * 128],
                            rhs=wkva_sb[:, k, 0:512],
                            start=(k == 0),
                            stop=(k == NT - 1),
                        )
                    # RMS sum-of-squares (ACT: Square only in phase A)
                    scr = f32p.tile([P, KLR], F32, tag="f32")
                    nc.scalar.activation(
                        scr[:], ps_c, AF.Square, accum_out=ssqs_sb[:, ti : ti + 1]
                    )
                    nc.vector.tensor_copy(kvcu_sb[:, ti, :], ps_c)

                # k_pe, directly transposed: [64(rope rows), t-chunk]
                ps_r = psL.tile([64, CH], F32, tag="lacc")
                for k in range(NT):
                    nc.tensor.matmul(
                        ps_r,
                        lhsT=wkva_sb[:, k, 512:576],
                        rhs=x_sb[:, k, :],
                        start=(k == 0),
                        stop=(k == NT - 1),
                    )
                # rope in [r, t] layout: rows 0:32 = re, 32:64 = im
                u1 = smallp.tile([32, CH], BF16, tag="sm32")
                u2 = smallp.tile([32, CH], BF16, tag="sm32")
                nc.vector.tensor_tensor(
                    u1[:], ps_r[0:32, :], cosS_sb[0:32, qs], ALU.mult
                )
                nc.vector.tensor_tensor(
                    u2[:], ps_r[32:64, :], sinS_sb[0:32, qs], ALU.mult
                )
                nc.vector.tensor_tensor(
                    kpeT_sb[0:32, qs], u1[:], u2[:], ALU.subtract
                )
                u3 = smallp.tile([32, CH], BF16, tag="sm32")
                u4 = smallp.tile([32, CH], BF16, tag="sm32")
                nc.vector.tensor_tensor(
                    u3[:], ps_r[0:32, :], sinS_sb[0:32, qs], ALU.mult
                )
                nc.vector.tensor_tensor(
                    u4[:], ps_r[32:64, :], cosS_sb[0:32, qs], ALU.mult
                )
                nc.vector.tensor_tensor(kpeT_sb[32:64, qs], u3[:], u4[:], ALU.add)

            # wo loads late, reusing the x-chunk slots
            woT_sb = xpool.tile([P, 4, DIM], BF16, tag="x")
            nc.sync.dma_start(woT_sb[:], woT)

            # ================= phase B: normalize + absorb =================
            # rms = sqrt(ssq/KLR + eps); rin = 1/rms  (one op for all tiles)
            rms_sb = smallp.tile([P, NT], F32, tag="smNT")
            nc.scalar.activation(
                rms_sb[:], ssqs_sb[:], AF.Sqrt, bias=eps_sb[:], scale=1.0 / KLR
            )
            rin_sb = smallp.tile([P, NT], F32, tag="smNT")
            nc.vector.reciprocal(rin_sb[:], rms_sb[:])
            # duplicate k_peT rope rows into partitions 64:128 (heads at base 64)
            nc.sync.dma_start(kpeT_sb[64:128, :], kpeT_sb[0:64, :])

            # normalize kv_c and transpose to [c, t]
            for ti in range(NT):
                tsl = slice(ti * 128, (ti + 1) * 128)
                kvn = smallp.tile([P, KLR], BF16, tag="sm")
                nc.vector.tensor_scalar_mul(
                    kvn[:], kvcu_sb[:, ti, :], rin_sb[:, ti : ti + 1]
                )
                for cs in range(4):
                    pool_t = psC if cs % 2 == 0 else psL
                    tp = pool_t.tile(
                        [P, P], BF16, tag="aux" if cs % 2 == 0 else "lacc"
                    )
                    nc.tensor.transpose(
                        tp, kvn[:, cs * 128 : (cs + 1) * 128], ident_sb[:]
                    )
                    nc.vector.tensor_copy(kvcT_sb[:, cs, tsl], tp)

            # k_abs[h] = wbk_h @ kv_c^T  ([d, t], per head, 512-wide t chunks)
            kabs_sb = wqpool.tile([P, HG, S], BF16, tag="wq")
            vabsT_sb = wkpool.tile([P, NT, HG * VHD], BF16, tag="wk")
            for hh in range(HG):
                for q2 in range(NCH):
                    qs2 = slice(q2 * CH, (q2 + 1) * CH)
                    ps = psA.tile([P, CH], F32, tag="oacc")
                    for cs in range(4):
                        nc.tensor.matmul(
                            ps,
                            lhsT=wbkT_sb[:, cs, hh, :],
                            rhs=kvcT_sb[:, cs, qs2],
                            start=(cs == 0),
                            stop=(cs == 3),
                        )
                    nc.vector.tensor_copy(kabs_sb[:, hh, qs2], ps)
            # v_absT = kv_c @ wbv^T  ([t, (h,d)], all 4 heads in one 512-wide go)
            for ti in range(NT):
                tsl = slice(ti * 128, (ti + 1) * 128)
                ps = psB.tile([P, HG * VHD], F32, tag="mm")
                for cs in range(4):
                    nc.tensor.matmul(
                        ps,
                        lhsT=kvcT_sb[:, cs, tsl],
                        rhs=wbvT_sb[:, cs, :],
                        start=(cs == 0),
                        stop=(cs == 3),
                    )
                nc.vector.tensor_copy(vabsT_sb[:, ti, :], ps)

            # ================= phase C: attention =================
            for qc in range(NCH):
                qs = slice(qc * CH, (qc + 1) * CH)
                ocat = ocp.tile([P, HG, CH], BF16, tag="oc")
                for hh in range(HG):
                    base = (hh % 2) * 64
                    jj = hh // 2
                    q_pe = qpe_sb[base : base + 64, jj, qs]
                    k_pe = kpeT_sb[base : base + 64, :]

                    oacc = psA.tile([P, CH], F32, tag="oacc")
                    l_ps = psL.tile([1, CH], F32, tag="lacc")
                    nti = 4 * qc + 4
                    # software-pipelined: emit sc(ti), then oacc(ti-1) so the
                    # PE streams scores while ACT runs the previous exp
                    exs = [None] * nti
                    offs = [max(0, t_ * 128 - qc * CH) for t_ in range(nti)]

                    def emit_oacc(t_):
                        off = offs[t_]
                        nc.tensor.matmul(
                            oacc[:, off:],
                            lhsT=vabsT_sb[:, t_, hh * VHD : (hh + 1) * VHD],
                            rhs=exs[t_][:, : CH - off],
                            start=(t_ == 0),
                            stop=(t_ == nti - 1),
                        )
                        nc.tensor.matmul(
                            l_ps[:, off:],
                            lhsT=onec_sb[:],
                            rhs=exs[t_][:, : CH - off],
                            start=(t_ == 0),
                            stop=(t_ == nti - 1),
                        )

                    for ti in range(nti):
                        tsl = slice(ti * 128, (ti + 1) * 128)
                        off = offs[ti]
                        nw = CH - off  # live sq columns (diag tiles shrink)
                        sc = psB.tile([P, CH], F32, tag="mm")
                        nc.tensor.matmul(
                            sc[:, :nw],
                            lhsT=kabs_sb[:, hh, tsl],
                            rhs=qnope_sb[:, hh, qc * CH + off : (qc + 1) * CH],
                            start=True,
                            stop=False,
                        )
                        nc.tensor.matmul(
                            sc[:, :nw],
                            lhsT=k_pe[:, tsl],
                            rhs=q_pe[:, off:],
                            start=False,
                            stop=True,
                        )
                        ex = expp.tile([P, CH], BF16, tag="exp")
                        nc.scalar.activation(ex[:, :nw], sc[:, :nw], AF.Exp)
                        if ti * 128 - qc * CH >= 0:  # diagonal: causal 0/1 mask
                            nc.vector.tensor_tensor(
                                ex[:, :nw],
                                ex[:, :nw],
                                mmst_sb[:, 384 : 384 + nw],
                                ALU.mult,
                            )
                        exs[ti] = ex
                        if ti > 0:
                            emit_oacc(ti - 1)
                    emit_oacc(nti - 1)
                    # 1/l broadcast across partitions via K=1 matmul
                    rl = f32p.tile([1, CH], F32R, tag="f32")
                    with nc.allow_low_precision(reason="1/l bcast via f32r matmul"):
                        nc.vector.reciprocal(rl[:], l_ps)
                    bc_ps = psC.tile([P, CH], F32, tag="aux")
                    nc.tensor.matmul(
                        bc_ps, lhsT=oner_sb[:], rhs=rl[:], start=True, stop=True
                    )
                    bc = f32p.tile([P, CH], F32, tag="f32")
                    nc.vector.tensor_copy(bc[:], bc_ps)
                    nc.vector.tensor_tensor(ocat[:, hh, :], oacc, bc[:], ALU.mult)
                # ---- output projection for this chunk ----
                for st2 in range(4):
                    st = qc * 4 + st2
                    for dc in range(4):
                        op = psB.tile([P, CH], F32, tag="mm")
                        for es in range(4):
                            nc.tensor.matmul(
                                op,
                                lhsT=ocat[:, es, st2 * 128 : (st2 + 1) * 128],
                                rhs=woT_sb[:, es, dc * CH : (dc + 1) * CH],
                                start=(es == 0),
                                stop=(es == 3),
                            )
                        ot = expp.tile([P, CH], BF16, tag="exp")
                        nc.vector.tensor_copy(ot[:], op)
                        nc.sync.dma_start(outp[:, st, dc * CH : (dc + 1) * CH], ot[:])
    return nc


# --- walrus in this container rejects >1 sem-wait per instruction; split ---
def _split_excess_waits(nc, max_waits=1):
    for f in nc.m.functions:
        for bb in f.blocks:
            if not any(
                i.sync_info is not None and len(i.sync_info.on_wait) > max_waits
                for i in bb.instructions
            ):
                continue
            new_insts = []
            for inst in bb.instructions:
                si = inst.sync_info
                if si is not None and len(si.on_wait) > max_waits:
                    waits = list(si.on_wait)
                    extra, keep = waits[:-max_waits], waits[-max_waits:]
                    for j in range(0, len(extra), max_waits):
                        nop = mybir.InstNoOp(
                            name=f"{inst.name}-wsplit-{j}", ins=[], outs=[]
                        )
                        nop.engine = inst.engine
                        nop.sync_info = mybir.SyncInfo(
                            on_wait=extra[j : j + max_waits], on_update=[]
                        )
                        new_insts.append(nop)
                    inst.sync_info = mybir.SyncInfo(
                        on_wait=keep, on_update=list(si.on_update)
                    )
                new_insts.append(inst)
            bb.instructions = new_insts


_NC = None


def _module():
    global _NC
    if _NC is None:
        nc = bass.Bass(
            "TRN2", target_bir_lowering=False, debug=False, num_devices=8
        )
        _emit(nc)
        _split_excess_waits(nc)
        _NC = nc
    return _NC


def _prep_core(core, x, wq, wkv_a, kv_norm_w, wkv_b, wo, fc, fs):
    """Build the per-core input map (numpy, host-side sharding + layouts)."""
    b, g = core // 4, core % 4
    heads = [4 * g + i for i in range(HG)]

    def bf(a):
        return np.ascontiguousarray(a.astype(NPBF16))

    m = {}
    xx = x[b]  # [S, DIM]
    m["xT"] = bf(xx.reshape(NCH, CH, NT, P).transpose(3, 0, 2, 1))

    rows_n = np.concatenate([h * QKHD + np.arange(NOPE) for h in heads])
    rows_lo = np.concatenate([h * QKHD + NOPE + 2 * np.arange(32) for h in heads])
    rows_hi = np.concatenate([h * QKHD + NOPE + 2 * np.arange(32) + 1 for h in heads])
    wqs = (wq * SCALE).astype(np.float32)
    for nm, rows in (("wqn", rows_n), ("wqlo", rows_lo), ("wqhi", rows_hi)):
        sel = wqs[rows]  # [M, DIM]
        m[nm] = bf(sel.T.reshape(NT, P, len(rows)).transpose(1, 0, 2))

    krows = np.concatenate(
        [np.arange(KLR), KLR + 2 * np.arange(32), KLR + 2 * np.arange(32) + 1]
    )
    m["wkva"] = bf(wkv_a[krows].T.reshape(NT, P, 576).transpose(1, 0, 2))

    wb = wkv_b.reshape(H, NOPE + VHD, KLR)
    wk = wb[heads, :NOPE, :] * kv_norm_w[None, None, :]  # [HG, d, c]
    m["wbkT"] = bf(
        wk.transpose(2, 0, 1).reshape(4, P, HG, NOPE).transpose(1, 0, 2, 3)
    )
    wv = wb[heads, NOPE:, :] * kv_norm_w[None, None, :]  # [HG, d, c]
    m["wbvT"] = bf(
        wv.transpose(2, 0, 1)
        .reshape(4, P, HG, VHD)
        .transpose(1, 0, 2, 3)
        .reshape(P, 4, HG * VHD)
    )

    wo_s = wo[:, 4 * g * VHD : 4 * (g + 1) * VHD]  # [DIM, 512]
    m["woT"] = bf(wo_s.T.reshape(4, P, DIM).transpose(1, 0, 2))

    m["cosS"] = bf(np.tile(fc.T, (4, 1)))
    m["sinS"] = bf(np.tile(fs.T, (4, 1)))

    pp = np.arange(P)[:, None]
    uu = np.arange(896)[None, :]
    m["mmst"] = bf((pp <= uu - 384).astype(np.float32))
    m["onec"] = bf(np.ones((P, 1), np.float32))
    m["oner"] = np.ones((1, P), np.float32)
    m["ident"] = bf(np.eye(P, dtype=np.float32))
    m["epsb"] = np.full((P, 1), EPS, np.float32)
    return m


def _make_in_maps(inputs):
    x = np.asarray(inputs["x"], np.float32)
    wq = np.asarray(inputs["wq"], np.float32)
    wkv_a = np.asarray(inputs["wkv_a"], np.float32)
    kv_norm_w = np.asarray(inputs["kv_norm_w"], np.float32)
    wkv_b = np.asarray(inputs["wkv_b"], np.float32)
    wo = np.asarray(inputs["wo"], np.float32)
    fc = np.asarray(inputs["freqs_cos"], np.float32)
    fs = np.asarray(inputs["freqs_sin"], np.float32)
    return [
        _prep_core(c, x, wq, wkv_a, kv_norm_w, wkv_b, wo, fc, fs) for c in range(8)
    ]


def _assemble(results):
    out = np.zeros((B, S, DIM), np.float32)
    for c in range(8):
        b = c // 4
        part = results[c]["outp"].astype(np.float32)  # [P, NT, DIM]
        out[b] += part.transpose(1, 0, 2).reshape(S, DIM)
    return out


def kernel(**inputs):
    nc = _module()
    in_maps = _make_in_maps(inputs)
    res = bass_utils.run_bass_kernel_spmd(nc, in_maps, core_ids=list(range(8)))
    return _assemble(res.results)


# revision 39
# speedup vs baseline: 1.4256x; 1.2892x over previous
"""MLA (multi-head latent attention) forward on 8 Trainium2 NeuronCores.

Sharding: tensor-parallel over heads (4 groups of 4 heads) x data-parallel
over batch (2), giving 8 cores. Every core computes the (small) latent
kv_c / k_pe projection over the full sequence itself — no collectives, no
cross-core coupling. wq / wkv_b rows and wo columns are sharded by head.
Each core produces a partial [S, DIM] output (its heads' contribution
through wo); the host sums the 4 head-group partials per batch element.

Key restructure vs a direct port of the reference: absorption is done on
the K/V side. Per head we precompute
    k_abs  = wbk_h @ kv_c^T        [d=128, S]
    v_absT = kv_c @ wbv_h^T        [S, d=128]
so the score matmul contracts over d=128 (1 matmul / 128-key-tile) and the
o accumulation contracts over the key tile (1 matmul), instead of
contracting over the 512-wide latent each time.

Per-core dataflow is "transposed" (feature dims on SBUF partitions,
sequence on the free dim):
  phase A (per 512-seq chunk): qT = wq_g @ x^T; kv = x @ wkv_a^T with
    RMS sum-of-squares accumulated (ACT does only Square); k_peT computed
    directly transposed and roped in [r, t] layout.
  phase B: rsqrt of mean-square (one ACT Sqrt + one DVE reciprocal for all
    tiles), normalize kv_c, PE-transpose to kv_cT, build k_abs / v_absT.
  phase C (per chunk, per head): scoresT[t, sq] = k_abs^T q_nope +
    k_peT^T q_pe; exp (no max subtraction — scores are O(1)); causal 0/1
    mask on diagonal tiles; oacc[d, sq] += v_absT^T exp; l via ones-row
    matmul; normalize by broadcast(1/l); out projection through woT.
"""

import numpy as np
import ml_dtypes

import concourse.bass as bass
import concourse.tile as tile
import concourse.mybir as mybir
from concourse import bass_utils

BF16 = mybir.dt.bfloat16
F32 = mybir.dt.float32
F32R = mybir.dt.float32r
AF = mybir.ActivationFunctionType
ALU = mybir.AluOpType
NPBF16 = ml_dtypes.bfloat16

B, S, DIM, H = 2, 2048, 2048, 16
NOPE, ROPE, VHD, KLR = 128, 64, 128, 512
QKHD = NOPE + ROPE
SCALE = QKHD ** -0.5
EPS = 1.1920929e-07
P = 128
HG = 4            # heads per core
CH = 512          # sequence chunk (matmul free dim)
NCH = S // CH     # 4 chunks
NT = S // P       # 16 tiles of 128


def _emit(nc):
    dt = nc.dram_tensor
    xT = dt("xT", [P, NCH, NT, CH], BF16, kind="ExternalInput").ap()
    wqn = dt("wqn", [P, NT, 512], BF16, kind="ExternalInput").ap()
    wqlo = dt("wqlo", [P, NT, 128], BF16, kind="ExternalInput").ap()
    wqhi = dt("wqhi", [P, NT, 128], BF16, kind="ExternalInput").ap()
    wkva = dt("wkva", [P, NT, 576], BF16, kind="ExternalInput").ap()
    wbkT = dt("wbkT", [P, 4, HG, NOPE], BF16, kind="ExternalInput").ap()
    wbvT = dt("wbvT", [P, 4, HG * VHD], BF16, kind="ExternalInput").ap()
    woT = dt("woT", [P, 4, DIM], BF16, kind="ExternalInput").ap()
    cosS = dt("cosS", [P, S], BF16, kind="ExternalInput").ap()
    sinS = dt("sinS", [P, S], BF16, kind="ExternalInput").ap()
    mmst = dt("mmst", [P, 896], BF16, kind="ExternalInput").ap()
    onec = dt("onec", [P, 1], BF16, kind="ExternalInput").ap()
    oner = dt("oner", [1, P], F32R, kind="ExternalInput").ap()
    ident = dt("ident", [P, P], BF16, kind="ExternalInput").ap()
    epsb = dt("epsb", [P, 1], F32, kind="ExternalInput").ap()
    outp = dt("outp", [P, NT, DIM], BF16, kind="ExternalOutput").ap()

    with tile.TileContext(nc) as tc:
        from contextlib import ExitStack

        with ExitStack() as ctx:
            ec = ctx.enter_context
            const = ec(tc.tile_pool(name="const", bufs=1))
            # 1-buf pools whose space is reused by a phase-B tensor after the
            # phase-A weight's last read
            wqpool = ec(tc.tile_pool(name="wqpool", bufs=1))
            wkpool = ec(tc.tile_pool(name="wkpool", bufs=1))
            xpool = ec(tc.tile_pool(name="xpool", bufs=2))
            ocp = ec(tc.tile_pool(name="ocp", bufs=2))
            expp = ec(tc.tile_pool(name="expp", bufs=5))
            f32p = ec(tc.tile_pool(name="f32p", bufs=3))
            smallp = ec(tc.tile_pool(name="smallp", bufs=7))

            psA = ec(tc.tile_pool(name="psA", bufs=3, space="PSUM"))
            psB = ec(tc.tile_pool(name="psB", bufs=2, space="PSUM"))
            psL = ec(tc.tile_pool(name="psL", bufs=1, space="PSUM"))
            psC = ec(tc.tile_pool(name="psC", bufs=2, space="PSUM"))

            # ---- resident weights/tables ----
            # First-needed data first: x chunk 0 + wqn, interleaved per
            # k-slice so the k=0 projection group unblocks early.
            # wqn is m-major [P, 4, NT, 128] so group m=0 is fully loaded
            # after x0 + 0.5MB instead of x0 + 2MB
            wqn_sb = wqpool.tile([P, NT, 512], BF16, tag="wq")
            x0_sb = xpool.tile([P, NT, CH], BF16, tag="x")
            for k4 in range(0, NT, 4):
                nc.sync.dma_start(
                    x0_sb[:, k4 : k4 + 4, :], xT[:, 0, k4 : k4 + 4, :]
                )
                nc.sync.dma_start(
                    wqn_sb[:, k4 : k4 + 4, :], wqn[:, k4 : k4 + 4, :]
                )
            wqlo_sb = const.tile([P, NT, 128], BF16, tag="wqlo")
            nc.sync.dma_start(wqlo_sb[:], wqlo)
            wqhi_sb = const.tile([P, NT, 128], BF16, tag="wqhi")
            nc.sync.dma_start(wqhi_sb[:], wqhi)
            cosS_sb = const.tile([P, S], BF16, tag="cosS")
            nc.sync.dma_start(cosS_sb[:], cosS)
            sinS_sb = const.tile([P, S], BF16, tag="sinS")
            nc.sync.dma_start(sinS_sb[:], sinS)
            wkva_sb = wkpool.tile([P, NT, 576], BF16, tag="wk")
            for k in range(NT):
                nc.sync.dma_start(wkva_sb[:, k, :], wkva[:, k, :])
            wbkT_sb = const.tile([P, 4, HG, NOPE], BF16, tag="wbkT")
            nc.sync.dma_start(wbkT_sb[:], wbkT)
            wbvT_sb = const.tile([P, 4, HG * VHD], BF16, tag="wbvT")
            nc.sync.dma_start(wbvT_sb[:], wbvT)
            mmst_sb = const.tile([P, 896], BF16, tag="mmst")
            nc.sync.dma_start(mmst_sb[:], mmst)
            onec_sb = const.tile([P, 1], BF16, tag="onec")
            nc.sync.dma_start(onec_sb[:], onec)
            oner_sb = const.tile([1, P], F32R, tag="oner")
            nc.sync.dma_start(oner_sb[:], oner)
            ident_sb = const.tile([P, P], BF16, tag="ident")
            nc.sync.dma_start(ident_sb[:], ident)
            eps_sb = const.tile([P, 1], F32, tag="epsb")
            nc.sync.dma_start(eps_sb[:], epsb)

            # ---- persistent activations ----
            qnope_sb = const.tile([P, HG, S], BF16, tag="qnope")
            # head h rope operand: partitions (h%2)*64 + [0,64), index h//2
            qpe_sb = const.tile([P, 2, S], BF16, tag="qpe")
            kvcu_sb = const.tile([P, NT, KLR], BF16, tag="kvcu")  # un-normed
            ssqs_sb = const.tile([P, NT], F32, tag="ssqs")
            kvcT_sb = const.tile([P, 4, S], BF16, tag="kvcT")
            kpeT_sb = const.tile([P, S], BF16, tag="kpeT")  # dup rows 64:128
            # kabs / vabsT are allocated in phase B from the wq / wkva pools
            # (same space, reused after the projection weights' last read)

            # ================= phase A: projections =================
            for q in range(NCH):
                qs = slice(q * CH, (q + 1) * CH)
                if q == 0:
                    x_sb = x0_sb
                    xnext = xpool.tile([P, NT, CH], BF16, tag="x")
                    nc.sync.dma_start(xnext[:], xT[:, 1])
                else:
                    x_sb = xnext
                    if q < NCH - 1:
                        xnext = xpool.tile([P, NT, CH], BF16, tag="x")
                        nc.sync.dma_start(xnext[:], xT[:, q + 1])

                # q projections: 6 m-tiles (4 nope heads + rope lo + hi),
                # m-outer / k-inner so 3 PSUM banks rotate over the groups
                lo_t = smallp.tile([P, CH], BF16, tag="sm")
                hi_t = smallp.tile([P, CH], BF16, tag="sm")

                def q_lhsT(m, k):
                    if m < HG:
                        return wqn_sb[:, k, m * 128 : (m + 1) * 128]
                    return (wqlo_sb if m == 4 else wqhi_sb)[:, k, :]

                for m in range(6):
                    ps = psA.tile([P, CH], F32, tag="A", name=f"q{m}")
                    for k in range(NT):
                        nc.tensor.matmul(
                            ps,
                            lhsT=q_lhsT(m, k),
                            rhs=x_sb[:, k, :],
                            start=(k == 0),
                            stop=(k == NT - 1),
                        )
                    if m < HG:
                        nc.vector.tensor_copy(qnope_sb[:, m, qs], ps)
                    elif m == 4:
                        nc.vector.tensor_copy(lo_t[:], ps)
                    else:
                        nc.vector.tensor_copy(hi_t[:], ps)
                # q rope on full-width tiles
                t1 = smallp.tile([P, CH], BF16, tag="sm")
                t2 = smallp.tile([P, CH], BF16, tag="sm")
                nc.vector.tensor_tensor(t1[:], lo_t[:], cosS_sb[:, qs], ALU.mult)
                nc.vector.tensor_tensor(t2[:], hi_t[:], sinS_sb[:, qs], ALU.mult)
                nc.vector.tensor_tensor(t1[:], t1[:], t2[:], ALU.subtract)
                t3 = smallp.tile([P, CH], BF16, tag="sm")
                t4 = smallp.tile([P, CH], BF16, tag="sm")
                nc.vector.tensor_tensor(t3[:], lo_t[:], sinS_sb[:, qs], ALU.mult)
                nc.vector.tensor_tensor(t4[:], hi_t[:], cosS_sb[:, qs], ALU.mult)
                nc.vector.tensor_tensor(t3[:], t3[:], t4[:], ALU.add)
                # repack to per-head contiguous [re;im] via SBUF->SBUF DMA
                for hh in range(HG):
                    base = (hh % 2) * 64
                    j = hh // 2
                    nc.sync.dma_start(
                        qpe_sb[base : base + 32, j, qs], t1[hh * 32 : (hh + 1) * 32, :]
                    )
                    nc.sync.dma_start(
                        qpe_sb[base + 32 : base + 64, j, qs],
                        t3[hh * 32 : (hh + 1) * 32, :],
                    )

                # kv_c projection for this chunk's 4 t-tiles ([t, c] layout)
                for st2 in range(4):
                    ti = q * 4 + st2
                    ps_c = psA.tile([P, KLR], F32, tag="A")
                    for k in range(NT):
                        nc.tensor.matmul(
                            ps_c,
                            lhsT=x_sb[:, k, st2 * 128 : (st2 + 1) * 128],
                            rhs=wkva_sb[:, k, 0:512],
                            start=(k == 0),
                            stop=(k == NT - 1),
                        )
                    # RMS sum-of-squares (ACT: Square only in phase A)
                    scr = f32p.tile([P, KLR], F32, tag="f32")
                    nc.scalar.activation(
                        scr[:], ps_c, AF.Square, accum_out=ssqs_sb[:, ti : ti + 1]
                    )
                    nc.vector.tensor_copy(kvcu_sb[:, ti, :], ps_c)

                # k_pe, directly transposed: [64(rope rows), t-chunk]
                ps_r = psL.tile([64, CH], F32, tag="lacc")
                for k in range(NT):
                    nc.tensor.matmul(
                        ps_r,
                        lhsT=wkva_sb[:, k, 512:576],
                        rhs=x_sb[:, k, :],
                        start=(k == 0),
                        stop=(k == NT - 1),
                    )
                # rope in [r, t] layout: rows 0:32 = re, 32:64 = im
                u1 = smallp.tile([32, CH], BF16, tag="sm32")
                u2 = smallp.tile([32, CH], BF16, tag="sm32")
                nc.vector.tensor_tensor(
                    u1[:], ps_r[0:32, :], cosS_sb[0:32, qs], ALU.mult
                )
                nc.vector.tensor_tensor(
                    u2[:], ps_r[32:64, :], sinS_sb[0:32, qs], ALU.mult
                )
                nc.vector.tensor_tensor(
                    kpeT_sb[0:32, qs], u1[:], u2[:], ALU.subtract
                )
                u3 = smallp.tile([32, CH], BF16, tag="sm32")
                u4 = smallp.tile([32, CH], BF16, tag="sm32")
                nc.vector.tensor_tensor(
                    u3[:], ps_r[0:32, :], sinS_sb[0:32, qs], ALU.mult
                )
                nc.vector.tensor_tensor(
                    u4[:], ps_r[32:64, :], cosS_sb[0:32, qs], ALU.mult
                )
                nc.vector.tensor_tensor(kpeT_sb[32:64, qs], u3[:], u4[:], ALU.add)

            # wo loads late, reusing the x-chunk slots
            woT_sb = xpool.tile([P, 4, DIM], BF16, tag="x")
            nc.sync.dma_start(woT_sb[:], woT)

            # ================= phase B: normalize + absorb =================
            # rms = sqrt(ssq/KLR + eps); rin = 1/rms  (one op for all tiles)
            rms_sb = smallp.tile([P, NT], F32, tag="smNT")
            nc.scalar.activation(
                rms_sb[:], ssqs_sb[:], AF.Sqrt, bias=eps_sb[:], scale=1.0 / KLR
            )
            rin_sb = smallp.tile([P, NT], F32, tag="smNT")
            nc.vector.reciprocal(rin_sb[:], rms_sb[:])
            # duplicate k_peT rope rows into partitions 64:128 (heads at base 64)
            nc.sync.dma_start(kpeT_sb[64:128, :], kpeT_sb[0:64, :])

            # normalize kv_c in place and transpose to [c, t]; scales run
            # 2 tiles ahead, PSUM drains alternate DVE / ACT so no single
            # engine serializes the chain
            kvns = [None] * NT

            def emit_scale(t_):
                # scale on ACT (Copy with per-partition scale) so DVE is free
                # for the transpose drains
                kvn = smallp.tile([P, KLR], BF16, tag="sm")
                nc.scalar.activation(
                    kvn[:], kvcu_sb[:, t_, :], AF.Copy,
                    scale=rin_sb[:, t_ : t_ + 1],
                )
                kvns[t_] = kvn

            emit_scale(0)
            emit_scale(1)
            for ti in range(NT):
                if ti + 2 < NT:
                    emit_scale(ti + 2)
                tsl = slice(ti * 128, (ti + 1) * 128)
                for cs in range(4):
                    pool_t = psC if cs % 2 == 0 else psB
                    tp = pool_t.tile([P, P], BF16, tag="C" if cs % 2 == 0 else "B")
                    nc.tensor.transpose(
                        tp, kvns[ti][:, cs * 128 : (cs + 1) * 128], ident_sb[:]
                    )
                    nc.vector.tensor_copy(kvcT_sb[:, cs, tsl], tp)

            # k_abs[h] = wbk_h @ kv_c^T  ([d, t], per head, 512-wide t chunks)
            kabs_sb = wqpool.tile([P, HG, S], BF16, tag="wq")
            vabsT_sb = wkpool.tile([P, NT, HG * VHD], BF16, tag="wk")
            for hh in range(HG):
                for q2 in range(NCH):
                    qs2 = slice(q2 * CH, (q2 + 1) * CH)
                    ps = psA.tile([P, CH], F32, tag="A")
                    for cs in range(4):
                        nc.tensor.matmul(
                            ps,
                            lhsT=wbkT_sb[:, cs, hh, :],
                            rhs=kvcT_sb[:, cs, qs2],
                            start=(cs == 0),
                            stop=(cs == 3),
                        )
                    nc.vector.tensor_copy(kabs_sb[:, hh, qs2], ps)
            # v_absT = kv_c @ wbv^T  ([t, (h,d)], all 4 heads in one 512-wide go)
            for ti in range(NT):
                tsl = slice(ti * 128, (ti + 1) * 128)
                ps = psB.tile([P, HG * VHD], F32, tag="B")
                for cs in range(4):
                    nc.tensor.matmul(
                        ps,
                        lhsT=kvcT_sb[:, cs, tsl],
                        rhs=wbvT_sb[:, cs, :],
                        start=(cs == 0),
                        stop=(cs == 3),
                    )
                nc.vector.tensor_copy(vabsT_sb[:, ti, :], ps)

            # ================= phase C: attention =================
            # Per (chunk, head): 3-deep score pipeline (sc on psA) hides the
            # exp latency; each head's finalize (1/l, broadcast, normalize)
            # is deferred into the NEXT head's score stream so the PE never
            # waits on the DVE reciprocal.
            def finalize_head(oacc, l_ps, ocat, hh):
                rl = f32p.tile([1, CH], F32R, tag="f32")
                with nc.allow_low_precision(reason="1/l bcast via f32r matmul"):
                    nc.vector.reciprocal(rl[:], l_ps)
                bc_ps = psL.tile([P, CH], F32, tag="lacc")
                nc.tensor.matmul(
                    bc_ps, lhsT=oner_sb[:], rhs=rl[:], start=True, stop=True
                )
                bc = f32p.tile([P, CH], F32, tag="f32")
                nc.vector.tensor_copy(bc[:], bc_ps)
                nc.vector.tensor_tensor(ocat[:, hh, :], oacc, bc[:], ALU.mult)

            for qc in range(NCH):
                qs = slice(qc * CH, (qc + 1) * CH)
                ocat = ocp.tile([P, HG, CH], BF16, tag="oc")
                pending = None
                tail = None
                for hh in range(HG):
                    base = (hh % 2) * 64
                    jj = hh // 2
                    q_pe = qpe_sb[base : base + 64, jj, qs]
                    k_pe = kpeT_sb[base : base + 64, :]

                    nti = 4 * qc + 4
                    exs = [None] * nti
                    offs = [max(0, t_ * 128 - qc * CH) for t_ in range(nti)]

                    def emit_sc(t_, hh=hh, exs=exs, offs=offs, qc=qc,
                                q_pe=q_pe, k_pe=k_pe):
                        tsl = slice(t_ * 128, (t_ + 1) * 128)
                        off = offs[t_]
                        nw = CH - off  # live sq columns (diag tiles shrink)
                        sc = psA.tile([P, CH], F32, tag="A")
                        nc.tensor.matmul(
                            sc[:, :nw],
                            lhsT=kabs_sb[:, hh, tsl],
                            rhs=qnope_sb[:, hh, qc * CH + off : (qc + 1) * CH],
                            start=True,
                            stop=False,
                        )
                        nc.tensor.matmul(
                            sc[:, :nw],
                            lhsT=k_pe[:, tsl],
                            rhs=q_pe[:, off:],
                            start=False,
                            stop=True,
                        )
                        ex = expp.tile([P, CH], BF16, tag="exp")
                        nc.scalar.activation(ex[:, :nw], sc[:, :nw], AF.Exp)
                        if t_ * 128 - qc * CH >= 0:  # diagonal: causal 0/1 mask
                            nc.vector.tensor_tensor(
                                ex[:, :nw],
                                ex[:, :nw],
                                mmst_sb[:, 384 : 384 + nw],
                                ALU.mult,
                            )
                        exs[t_] = ex

                    emit_sc(0)
                    if nti > 1:
                        emit_sc(1)
                    # previous head's LAST oacc + finalize run behind this
                    # head's first score groups, hiding the exp/reciprocal
                    # latency they wait on
                    if tail is not None:
                        tail()
                        tail = None
                    if pending is not None:
                        finalize_head(*pending)
                        pending = None
                    # oacc / l_ps allocated AFTER the previous head's finalize
                    # so the psL slot order is l(h), bc(h), l(h+1), ...
                    oacc = psB.tile([P, CH], F32, tag="B")
                    l_ps = psL.tile([1, CH], F32, tag="lacc")

                    def emit_oacc(t_, oacc=oacc, l_ps=l_ps, exs=exs,
                                  offs=offs, hh=hh, nti=nti):
                        off = offs[t_]
                        nc.tensor.matmul(
                            oacc[:, off:],
                            lhsT=vabsT_sb[:, t_, hh * VHD : (hh + 1) * VHD],
                            rhs=exs[t_][:, : CH - off],
                            start=(t_ == 0),
                            stop=(t_ == nti - 1),
                        )
                        nc.tensor.matmul(
                            l_ps[:, off:],
                            lhsT=onec_sb[:],
                            rhs=exs[t_][:, : CH - off],
                            start=(t_ == 0),
                            stop=(t_ == nti - 1),
                        )

                    for ti in range(nti - 1):
                        if ti + 2 < nti:
                            emit_sc(ti + 2)
                        emit_oacc(ti)
                    tail = (lambda f=emit_oacc, t_=nti - 1: f(t_))
                    pending = (oacc, l_ps, ocat, hh)
                # ---- output projection for this chunk ----
                # es 0-2 of each group are emitted before the es=3 matmul of
                # the previous group, so the last head's deferred finalize
                # (emitted between the first two partial groups) overlaps PE
                # work that doesn't depend on it.
                groups = [(st2, dc) for st2 in range(4) for dc in range(4)]
                ops = [None] * len(groups)

                def op_partial(i):
                    st2, dc = groups[i]
                    pool = psL if i % 3 == 2 else psC
                    op = pool.tile([P, CH], F32, tag="lacc" if i % 3 == 2 else "C")
                    ops[i] = op
                    for es in range(3):
                        nc.tensor.matmul(
                            op,
                            lhsT=ocat[:, es, st2 * 128 : (st2 + 1) * 128],
                            rhs=woT_sb[:, es, dc * CH : (dc + 1) * CH],
                            start=(es == 0),
                            stop=False,
                        )

                def op_final(i):
                    st2, dc = groups[i]
                    st = qc * 4 + st2
                    op = ops[i]
                    nc.tensor.matmul(
                        op,
                        lhsT=ocat[:, 3, st2 * 128 : (st2 + 1) * 128],
                        rhs=woT_sb[:, 3, dc * CH : (dc + 1) * CH],
                        start=False,
                        stop=True,
                    )
                    ot = expp.tile([P, CH], BF16, tag="exp")
                    nc.vector.tensor_copy(ot[:], op)
                    nc.sync.dma_start(outp[:, st, dc * CH : (dc + 1) * CH], ot[:])

                op_partial(0)
                tail()
                tail = None
                finalize_head(*pending)
                pending = None
                for i in range(len(groups)):
                    if i + 1 < len(groups):
                        op_partial(i + 1)
                    op_final(i)
    return nc


# --- walrus in this container rejects >1 sem-wait per instruction; split ---
def _split_excess_waits(nc, max_waits=1):
    for f in nc.m.functions:
        for bb in f.blocks:
            if not any(
                i.sync_info is not None and len(i.sync_info.on_wait) > max_waits
                for i in bb.instructions
            ):
                continue
            new_insts = []
            for inst in bb.instructions:
                si = inst.sync_info
                if si is not None and len(si.on_wait) > max_waits:
                    waits = list(si.on_wait)
                    extra, keep = waits[:-max_waits], waits[-max_waits:]
                    for j in range(0, len(extra), max_waits):
                        nop = mybir.InstNoOp(
                            name=f"{inst.name}-wsplit-{j}", ins=[], outs=[]
                        )
                        nop.engine = inst.engine
                        nop.sync_info = mybir.SyncInfo(
                            on_wait=extra[j : j + max_waits], on_update=[]
                        )
                        new_insts.append(nop)
                    inst.sync_info = mybir.SyncInfo(
                        on_wait=keep, on_update=list(si.on_update)
                    )
                new_insts.append(inst)
            bb.instructions = new_insts


_NC = None


def _module():
    global _NC
    if _NC is None:
        nc = bass.Bass(
            "TRN2", target_bir_lowering=False, debug=False, num_devices=8
        )
        _emit(nc)
        _split_excess_waits(nc)
        _NC = nc
    return _NC


def _prep_core(core, x, wq, wkv_a, kv_norm_w, wkv_b, wo, fc, fs):
    """Build the per-core input map (numpy, host-side sharding + layouts)."""
    b, g = core // 4, core % 4
    heads = [4 * g + i for i in range(HG)]

    def bf(a):
        return np.ascontiguousarray(a.astype(NPBF16))

    m = {}
    xx = x[b]  # [S, DIM]
    m["xT"] = bf(xx.reshape(NCH, CH, NT, P).transpose(3, 0, 2, 1))

    rows_n = np.concatenate([h * QKHD + np.arange(NOPE) for h in heads])
    rows_lo = np.concatenate([h * QKHD + NOPE + 2 * np.arange(32) for h in heads])
    rows_hi = np.concatenate([h * QKHD + NOPE + 2 * np.arange(32) + 1 for h in heads])
    wqs = (wq * SCALE).astype(np.float32)
    for nm, rows in (("wqn", rows_n), ("wqlo", rows_lo), ("wqhi", rows_hi)):
        sel = wqs[rows]  # [M, DIM]
        m[nm] = bf(sel.T.reshape(NT, P, len(rows)).transpose(1, 0, 2))

    krows = np.concatenate(
        [np.arange(KLR), KLR + 2 * np.arange(32), KLR + 2 * np.arange(32) + 1]
    )
    m["wkva"] = bf(wkv_a[krows].T.reshape(NT, P, 576).transpose(1, 0, 2))

    wb = wkv_b.reshape(H, NOPE + VHD, KLR)
    wk = wb[heads, :NOPE, :] * kv_norm_w[None, None, :]  # [HG, d, c]
    m["wbkT"] = bf(
        wk.transpose(2, 0, 1).reshape(4, P, HG, NOPE).transpose(1, 0, 2, 3)
    )
    wv = wb[heads, NOPE:, :] * kv_norm_w[None, None, :]  # [HG, d, c]
    m["wbvT"] = bf(
        wv.transpose(2, 0, 1)
        .reshape(4, P, HG, VHD)
        .transpose(1, 0, 2, 3)
        .reshape(P, 4, HG * VHD)
    )

    wo_s = wo[:, 4 * g * VHD : 4 * (g + 1) * VHD]  # [DIM, 512]
    m["woT"] = bf(wo_s.T.reshape(4, P, DIM).transpose(1, 0, 2))

    m["cosS"] = bf(np.tile(fc.T, (4, 1)))
    m["sinS"] = bf(np.tile(fs.T, (4, 1)))

    pp = np.arange(P)[:, None]
    uu = np.arange(896)[None, :]
    m["mmst"] = bf((pp <= uu - 384).astype(np.float32))
    m["onec"] = bf(np.ones((P, 1), np.float32))
    m["oner"] = np.ones((1, P), np.float32)
    m["ident"] = bf(np.eye(P, dtype=np.float32))
    m["epsb"] = np.full((P, 1), EPS, np.float32)
    return m


def _make_in_maps(inputs):
    x = np.asarray(inputs["x"], np.float32)
    wq = np.asarray(inputs["wq"], np.float32)
    wkv_a = np.asarray(inputs["wkv_a"], np.float32)
    kv_norm_w = np.asarray(inputs["kv_norm_w"], np.float32)
    wkv_b = np.asarray(inputs["wkv_b"], np.float32)
    wo = np.asarray(inputs["wo"], np.float32)
    fc = np.asarray(inputs["freqs_cos"], np.float32)
    fs = np.asarray(inputs["freqs_sin"], np.float32)
    return [
        _prep_core(c, x, wq, wkv_a, kv_norm_w, wkv_b, wo, fc, fs) for c in range(8)
    ]


def _assemble(results):
    out = np.zeros((B, S, DIM), np.float32)
    for c in range(8):
        b = c // 4
        part = results[c]["outp"].astype(np.float32)  # [P, NT, DIM]
        out[b] += part.transpose(1, 0, 2).reshape(S, DIM)
    return out


def kernel(**inputs):
    nc = _module()
    in_maps = _make_in_maps(inputs)
    res = bass_utils.run_bass_kernel_spmd(nc, in_maps, core_ids=list(range(8)))
    return _assemble(res.results)
